# revision 1
# baseline (speedup 1.0000x reference)
"""LocalRmsNorm Trainium2 kernel.

Problem: x (8, 16384, 256) f32 viewed as (b, h=128, w=128, d=256).
mean_sq = 7x7 zero-padded box mean of x^2 over (h, w); out = x / sqrt(eps + mean_sq) * weight.

Strategy (pure batch-parallel, one batch element per NeuronCore):
  - SBUF layout: partitions = h (128), free = (w, d) tiled by WT=16 w-columns.
  - sq = x^2 in fp16 on ScalarE (Square activation, cast on write).
  - Pair sums w2'[a] = sq[a] + sq[a+1] on VectorE (fp16, 2x mode).
  - 7x7 box sum entirely on the TensorEngine: box7[w'] = B_h @ (w2'[w'-3] +
    w2'[w'-1] + w2'[w'+1] + sq[w'+3]) where B_h is the [128,128] banded
    ones matrix handling the h-axis sum (zero padding free via band
    truncation). The four w-taps are PSUM-accumulating matmuls with shifted
    rhs access patterns; the band stays loaded as PE stationary weights.
  - inv = exp(-0.5 * ln(box/49 + eps)) on ScalarE (Rsqrt activation is
    banned for accuracy; Ln+Exp keeps the rsqrt off the critical DVE path).
  - out = x * inv (VectorE fp32), optional * weight (GpSimd) when weight != 1.
"""

import sys

if "/opt/trn_rl_repo" not in sys.path:
    sys.path.insert(0, "/opt/trn_rl_repo")

import numpy as np

H = 128          # h rows -> SBUF partitions
W = 128          # w columns
D = 256          # channels (free-dim innermost)
WT = 16          # w columns per tile
FT = WT * D      # free elems per tile (4096 f32)
CH = 2048        # psum / scalar-act chunk (f32 elems) = 8 w cols
EPS = 1e-7
KK = 49.0
NCORES = 8


def build_nc(apply_weight=False, n_wtiles=W // WT, inv_mode="lnexp",
             repeat=1):
    from contextlib import ExitStack

    import concourse.tile as tile
    from concourse import bacc, mybir

    dt = mybir.dt
    AF = mybir.ActivationFunctionType
    P = 128
    NT = n_wtiles
    Wl = NT * WT

    nc = bacc.Bacc("TRN2", target_bir_lowering=False)
    x_d = nc.dram_tensor("x", [P, Wl * D], dt.float32, kind="ExternalInput")
    band_d = nc.dram_tensor("band", [P, P], dt.float16, kind="ExternalInput")
    wrep_d = None
    if apply_weight:
        wrep_d = nc.dram_tensor("wrep", [P, FT], dt.float32, kind="ExternalInput")
    out_d = nc.dram_tensor("out", [P, Wl * D], dt.float32, kind="ExternalOutput")

    with ExitStack() as ctx:
        tc = ctx.enter_context(tile.TileContext(nc))
        xpool = ctx.enter_context(tc.tile_pool(name="x", bufs=3))
        sqpool = ctx.enter_context(tc.tile_pool(name="sq", bufs=3))
        w2pool = ctx.enter_context(tc.tile_pool(name="w2", bufs=4))
        tpool = ctx.enter_context(tc.tile_pool(name="t", bufs=2))
        invpool = ctx.enter_context(tc.tile_pool(name="inv", bufs=2))
        outpool = ctx.enter_context(tc.tile_pool(name="o", bufs=2))
        singles = ctx.enter_context(tc.tile_pool(name="s", bufs=1))
        psum = ctx.enter_context(tc.tile_pool(name="ps", bufs=2, space="PSUM"))

        band_t = singles.tile([P, P], dt.float16)
        nc.sync.dma_start(out=band_t[:, :], in_=band_d[:, :])
        eps_t = singles.tile([P, 1], dt.float32)
        nc.vector.memset(eps_t[:, :], EPS)
        zero_t = singles.tile([P, 1], dt.float32)
        nc.vector.memset(zero_t[:, :], 0.0)
        wrep_t = None
        if apply_weight:
            wrep_t = singles.tile([P, FT], dt.float32)
            nc.sync.dma_start(out=wrep_t[:, :], in_=wrep_d[:, :])

        x_tiles = [None] * NT
        sq_tiles = [None] * NT
        w2_tiles = [None] * (NT + 1)

        def w2_ap(a):
            # w2'[a] = sq[a] + sq[a+1], stored in tile m=(a+1)//WT col (a+1)%WT.
            # Returns the 2-col slice for global w pair {a, a+1}, or None if
            # that pair is entirely in the zero padding.
            m, j0 = divmod(a + 1, WT)
            if m < 0:
                return None
            return w2_tiles[m][:, j0 * D:(j0 + 2) * D]

        def emit_pe(i):
            inv_t = invpool.tile([P, FT], dt.float32)
            for half in range(2):
                ps = psum.tile([P, CH], dt.float32)
                for q in range(CH // 512):
                    g = i * WT + half * (CH // D) + 2 * q  # first out w col
                    po = ps[:, q * 512:(q + 1) * 512]
                    entries = [(po, w2_ap(g - 1))]  # always in-range
                    a3 = w2_ap(g - 3)
                    if a3 is not None:
                        entries.append((po, a3))
                    # sq tap at +3: sources {g+3, g+4}, may straddle tiles
                    m0, j0 = divmod(g + 3, WT)
                    m1, j1 = divmod(g + 4, WT)
                    if m0 == m1:
                        if m0 < NT:
                            entries.append(
                                (po, sq_tiles[m0][:, j0 * D:(j0 + 2) * D]))
                    else:
                        if m0 < NT:
                            entries.append((ps[:, q * 512:q * 512 + D],
                                            sq_tiles[m0][:, j0 * D:(j0 + 1) * D]))
                        if m1 < NT:
                            entries.append((ps[:, q * 512 + D:(q + 1) * 512],
                                            sq_tiles[m1][:, j1 * D:(j1 + 1) * D]))
                    entries.append((po, w2_ap(g + 1)))  # always in-range
                    n = len(entries)
                    for k, (o, r) in enumerate(entries):
                        nc.tensor.matmul(o, band_t[:, :], r,
                                         start=(k == 0), stop=(k == n - 1))
                half_sl = inv_t[:, half * CH:(half + 1) * CH]
                if inv_mode == "lnexp":
                    t_t = tpool.tile([P, CH], dt.float32)
                    nc.scalar.activation(t_t[:, :], ps[:, :], AF.Ln,
                                         bias=eps_t[:, :], scale=1.0 / KK)
                    nc.scalar.activation(half_sl, t_t[:, :], AF.Exp,
                                         bias=zero_t[:, :], scale=-0.5)
                else:  # sqrt + vector reciprocal
                    t_t = tpool.tile([P, CH], dt.float32)
                    nc.scalar.activation(t_t[:, :], ps[:, :], AF.Sqrt,
                                         bias=eps_t[:, :], scale=1.0 / KK)
                    nc.vector.reciprocal(half_sl, t_t[:, :])
            if apply_weight:
                nc.gpsimd.tensor_mul(inv_t[:, :], inv_t[:, :], wrep_t[:, :])
            o_t = outpool.tile([P, FT], dt.float32)
            nc.vector.tensor_mul(o_t[:, :], x_tiles[i][:, :], inv_t[:, :])
            nc.sync.dma_start(out=out_d[:, i * FT:(i + 1) * FT], in_=o_t[:, :])

        def body():
            for i in range(NT):
                x_t = xpool.tile([P, FT], dt.float32)
                nc.sync.dma_start(out=x_t[:, :],
                                  in_=x_d[:, i * FT:(i + 1) * FT])
                x_tiles[i] = x_t
                sq_t = sqpool.tile([P, FT], dt.float16)
                nc.scalar.square(sq_t[:, :], x_t[:, :])
                sq_tiles[i] = sq_t
                w2_t = w2pool.tile([P, FT], dt.float16)
                if i == 0:
                    # w2'[-1] = sq[-1] + sq[0] = sq[0]
                    nc.vector.tensor_copy(w2_t[:, 0:D], sq_t[:, 0:D])
                else:
                    nc.vector.tensor_add(w2_t[:, 0:D],
                                         sq_tiles[i - 1][:, (WT - 1) * D:WT * D],
                                         sq_t[:, 0:D])
                nc.vector.tensor_add(w2_t[:, D:FT],
                                     sq_t[:, 0:(WT - 1) * D],
                                     sq_t[:, D:FT])
                w2_tiles[i] = w2_t
                if i >= 1:
                    emit_pe(i - 1)

            # tail: w2'[W-1] = sq[W-1] + 0, w2'[W] = 0
            w2tail = singles.tile([P, 2 * D], dt.float16)
            nc.vector.tensor_copy(w2tail[:, 0:D],
                                  sq_tiles[NT - 1][:, (WT - 1) * D:WT * D])
            nc.vector.memset(w2tail[:, D:2 * D], 0.0)
            w2_tiles[NT] = w2tail
            emit_pe(NT - 1)

        if repeat == 1:
            body()
        else:
            with tc.For_i(0, repeat, 1,
                          hint_engines=(mybir.EngineType.PE,
                                        mybir.EngineType.Activation)):
                body()

    nc.finalize()
    return nc


_NC_CACHE = {}


def _get_nc(apply_weight):
    key = apply_weight
    if key not in _NC_CACHE:
        _NC_CACHE[key] = build_nc(apply_weight=apply_weight)
    return _NC_CACHE[key]


def _band_np():
    idx = np.arange(H)
    return (np.abs(idx[:, None] - idx[None, :]) <= 3).astype(np.float16)


LAST_RESULT = None


def kernel(x, weight, trace=False):
    global LAST_RESULT
    x = np.ascontiguousarray(np.asarray(x), dtype=np.float32)
    weight = np.asarray(weight, dtype=np.float32).reshape(D)
    assert x.shape == (NCORES, H * W, D), x.shape
    apply_w = not bool(np.all(weight == np.float32(1.0)))
    nc = _get_nc(apply_w)
    band = _band_np()
    in_maps = []
    for c in range(NCORES):
        m = {"x": x[c].reshape(H, W * D), "band": band}
        if apply_w:
            m["wrep"] = np.ascontiguousarray(
                np.tile(weight, (H, WT))).astype(np.float32)
        in_maps.append(m)
    from concourse.bass_utils import run_bass_kernel_spmd

    res = run_bass_kernel_spmd(nc, in_maps, core_ids=list(range(NCORES)),
                               trace=trace)
    LAST_RESULT = res
    out = np.stack([r["out"].reshape(H * W, D) for r in res.results], axis=0)
    return np.ascontiguousarray(out, dtype=np.float32)

